# revision 20
# baseline (speedup 1.0000x reference)
"""Trainium2 Bass kernel for rank-1-projection attention.

Computation (reference, fp32):
    q = x_q @ WQ            [512,512,256]@[256] -> [512,512]
    k = x_k @ WK
    v = x_v @ WV
    y = softmax(q @ k, axis=-1) @ v     -> [512,512]

Strategy (v2): data-parallel over the leading N axis (64 rows/core x 8
cores).  The host pre-transposes each core's x slabs to d-major fp16
([256, 32768]), so the rank-1 projections run entirely on the tensor
engine: each [128 d, 128 rows] chunk is loaded as the stationary lhsT
and multiplied by the W-half [128, 1] moving operand, producing one
fp32 PSUM column per chunk (~30 ns each measured).  DVE/GpSimd do no
bulk work; per-core HBM traffic halves vs fp32 (48 MB -> ~140 us DMA
floor, the roofline).  k/v projections are re-tiled on-chip ([i, m]
rows), AllGathered in fp16, and the tiny attention tail runs fp16 on
the PE with fp32 PSUM accumulation.
"""

import numpy as np

import concourse.bass as bass
import concourse.mybir as mybir
import concourse.tile as tile
from concourse import bacc
from concourse.bass_utils import run_bass_kernel_spmd
from concourse.masks import make_identity

N = 512          # attention size (rows/cols)
D = 256          # projection dim
CORES = 8
NL = N // CORES  # 64 leading rows per core
R = NL * N       # 32768 projection rows per tensor per core
RNG = 4096       # rows per DMA tile ([128, RNG] fp16 = 1 MB)
NRG = R // RNG   # 8 ranges per tensor
CPT = RNG // 128  # 32 chunks of 128 rows per tile

F32 = mybir.dt.float32
F16 = mybir.dt.float16

_CACHE = {}


def _build():
    if "nc" in _CACHE:
        return _CACHE["nc"]

    nc = bacc.Bacc(
        "TRN2", target_bir_lowering=False, debug=False, num_devices=CORES
    )

    xkt = nc.dram_tensor("xkt", [D, R], F16, kind="ExternalInput")
    xvt = nc.dram_tensor("xvt", [D, R], F16, kind="ExternalInput")
    xqt = nc.dram_tensor("xqt", [D, R], F16, kind="ExternalInput")
    wall = nc.dram_tensor("wall", [128, 6], F16, kind="ExternalInput")
    yout = nc.dram_tensor("yout", [NL, N], F32, kind="ExternalOutput")

    with tile.TileContext(nc) as tc:
        with (
            tc.tile_pool(name="consts", bufs=1) as consts,
            tc.tile_pool(name="xs", bufs=6) as xs_pool,
            tc.tile_pool(name="psum", bufs=1, space="PSUM") as psum_pool,
            tc.tile_pool(name="dram", bufs=1, space="DRAM") as dram_pool,
        ):
            w_t = consts.tile([128, 6], F16)
            nc.scalar.dma_start(w_t[:], wall[:])
            ident = consts.tile([128, 128], F32)
            make_identity(nc, ident[:])

            # fp32 psum accumulators, [b%128, (b//128)*64 + i] layout:
            # ps[p, bb*64 + a] = proj value of slab row a*512 + bb*128 + p
            ps = {
                t: psum_pool.tile([128, 4 * NL], F32, tag=f"ps{t}", name=f"ps{t}")
                for t in ("k", "v", "q")
            }

            def project(x_dram, widx, dest):
                for rg in range(NRG):
                    tiles = []
                    for h in (0, 1):
                        xt = xs_pool.tile([128, RNG], F16, tag="xt", name="xt")
                        # alternate hwdge rings to hide per-DMA latency gaps
                        ring = nc.sync if h == 0 else nc.scalar
                        ring.dma_start(
                            xt[:],
                            x_dram[h * 128 : (h + 1) * 128,
                                   rg * RNG : (rg + 1) * RNG],
                        )
                        tiles.append(xt)
                    for j in range(CPT):
                        # slab rows rg*RNG + j*128 ... +128:
                        # a = rg*(RNG//512) + j//4, b-block bb = j%4
                        # ->  psum column bb*64 + a
                        col = (j % 4) * NL + rg * (RNG // N) + j // 4
                        for h in (0, 1):
                            nc.tensor.matmul(
                                dest[:, col : col + 1],
                                lhsT=tiles[h][:, j * 128 : (j + 1) * 128],
                                rhs=w_t[:, 2 * widx + h : 2 * widx + h + 1],
                                start=(h == 0),
                                stop=(h == 1),
                            )

            # re-tile a projection psum [128, 256] into [a, b] rows (fp16)
            def pack_rows(src_ps, dst, dst_off):
                sbt = consts.tile([128, 4 * NL], F32, name=f"sbt{dst_off}")
                nc.scalar.activation(
                    sbt[:], src_ps[:], mybir.ActivationFunctionType.Copy
                )
                for bb in range(4):
                    pt = psum_pool.tile([NL, 128], F32, tag="tp", bufs=1, name="pt")
                    nc.tensor.transpose(
                        pt[:], sbt[:, bb * NL : (bb + 1) * NL], ident[:]
                    )
                    nc.vector.tensor_copy(
                        out=dst[:, dst_off + bb * 128 : dst_off + (bb + 1) * 128],
                        in_=pt[:],
                    )

            # ---- k then v, each gathered right away so the collective
            # overlaps the remaining x streams ----
            kv_loc = consts.tile([NL, 2 * N], F16)
            cc_in_k = dram_pool.tile([NL, N], F16)
            cc_in_v = dram_pool.tile([NL, N], F16)
            cc_out_k = dram_pool.tile([N, N], F16, addr_space="Shared")
            cc_out_v = dram_pool.tile([N, N], F16, addr_space="Shared")

            project(xkt, 1, ps["k"])
            pack_rows(ps["k"], kv_loc, 0)
            nc.gpsimd.dma_start(cc_in_k[:], kv_loc[:, 0:N])
            nc.gpsimd.collective_compute(
                "AllGather",
                mybir.AluOpType.bypass,
                replica_groups=[list(range(CORES))],
                ins=[cc_in_k[:].opt()],
                outs=[cc_out_k[:].opt()],
            )

            project(xvt, 2, ps["v"])
            pack_rows(ps["v"], kv_loc, N)
            nc.gpsimd.dma_start(cc_in_v[:], kv_loc[:, N : 2 * N])
            nc.gpsimd.collective_compute(
                "AllGather",
                mybir.AluOpType.bypass,
                replica_groups=[list(range(CORES))],
                ins=[cc_in_v[:].opt()],
                outs=[cc_out_v[:].opt()],
            )

            # ---- q projection (overlaps with the AllGather) ----
            project(xqt, 0, ps["q"])
            # q stays in [m%128, (m//128)*64 + i] layout: lhsT blocks for the
            # qk matmul are direct [64, 64] slices of it
            q_sbT = consts.tile([128, 4 * NL], F16)
            nc.scalar.activation(
                q_sbT[:], ps["q"][:], mybir.ActivationFunctionType.Copy
            )

            # gathered k/v rows, two ranks per [128, N] tile:
            # k_sb[b][64*(r%2) + m_local, j] = k row of rank r = 2b + (r%2)
            k_sb = [consts.tile([128, N], F16, name=f"ksb{b}") for b in range(4)]
            v_sb = [consts.tile([128, N], F16, name=f"vsb{b}") for b in range(4)]
            for b in range(4):
                nc.gpsimd.dma_start(
                    k_sb[b][:], cc_out_k[b * 128 : (b + 1) * 128, :]
                )
                nc.gpsimd.dma_start(
                    v_sb[b][:], cc_out_v[b * 128 : (b + 1) * 128, :]
                )

            # ---- attention tail ----
            # q_sbT[:, b*64:(b+1)*64] is q[i, m] transposed for m-block b
            # (128 m rows = gathered ranks 2b, 2b+1) -> 4 full-K matmuls
            py = psum_pool.tile([NL, N], F32, tag="mm", name="py")
            for b in range(4):
                nc.tensor.matmul(
                    py[:], lhsT=q_sbT[:, b * NL : (b + 1) * NL], rhs=k_sb[b][:],
                    start=(b == 0), stop=(b == 3),
                )

            neg_mx = consts.tile([NL, 1], F32)
            nc.vector.tensor_reduce(
                out=neg_mx[:], in_=py[:], axis=mybir.AxisListType.X,
                op=mybir.AluOpType.max, negate=True,
            )
            s_sb = consts.tile([NL, N], F32)
            sumexp = consts.tile([NL, 1], F32)
            nc.scalar.activation(
                s_sb[:], py[:], mybir.ActivationFunctionType.Exp,
                bias=neg_mx[:], scale=1.0, accum_out=sumexp[:],
            )
            rsum = consts.tile([NL, 1], F32)
            nc.vector.reciprocal(rsum[:], sumexp[:])

            # st2[b]: transposed softmax blocks for ranks 2b, 2b+1 stacked
            st2 = [consts.tile([128, NL], F16, name=f"st{b}") for b in range(4)]
            for b in range(4):
                for half in (0, 1):
                    r = 2 * b + half
                    pt2 = psum_pool.tile([NL, NL], F32, tag="tp2", bufs=1, name="pt2")
                    nc.tensor.transpose(
                        pt2[:],
                        s_sb[:, r * NL : (r + 1) * NL],
                        ident[:NL, :NL],
                    )
                    nc.vector.tensor_copy(
                        out=st2[b][NL * half : NL * half + NL, :], in_=pt2[:]
                    )

            po = psum_pool.tile([NL, N], F32, tag="mm2", name="po")
            for b in range(4):
                nc.tensor.matmul(
                    po[:], lhsT=st2[b][:], rhs=v_sb[b][:],
                    start=(b == 0), stop=(b == 3),
                )

            out_sb = consts.tile([NL, N], F32)
            nc.vector.tensor_scalar_mul(out_sb[:], po[:], rsum[:])
            nc.sync.dma_start(yout[:], out_sb[:])

    nc.compile()
    _CACHE["nc"] = nc
    return nc


def _make_in_maps(inputs):
    x_q = np.asarray(inputs["x_q"], dtype=np.float32)
    x_k = np.asarray(inputs["x_k"], dtype=np.float32)
    x_v = np.asarray(inputs["x_v"], dtype=np.float32)
    w_all = np.stack(
        [
            np.asarray(inputs["WQ"], dtype=np.float32),
            np.asarray(inputs["WK"], dtype=np.float32),
            np.asarray(inputs["WV"], dtype=np.float32),
        ],
        axis=1,
    ).reshape(2, 128, 3).transpose(1, 2, 0).reshape(128, 6)  # [p, 2*tensor+half]
    w_all = np.ascontiguousarray(w_all).astype(np.float16)
    in_maps = []
    for r in range(CORES):
        sl = slice(r * NL, (r + 1) * NL)
        in_maps.append(
            {
                "xqt": x_q[sl].reshape(R, D).T.astype(np.float16),
                "xkt": x_k[sl].reshape(R, D).T.astype(np.float16),
                "xvt": x_v[sl].reshape(R, D).T.astype(np.float16),
                "wall": w_all,
            }
        )
    return in_maps


def _run(inputs, trace=False):
    nc = _build()
    res = run_bass_kernel_spmd(
        nc, _make_in_maps(inputs), core_ids=list(range(CORES)), trace=trace
    )
    out = np.concatenate(
        [res.results[r]["yout"] for r in range(CORES)], axis=0
    ).astype(np.float32)
    return out, res


def kernel(**inputs):
    out, _ = _run(inputs)
    return out


# revision 21
# speedup vs baseline: 1.0756x; 1.0756x over previous
"""Trainium2 Bass kernel for rank-1-projection attention.

Computation (reference, fp32):
    q = x_q @ WQ            [512,512,256]@[256] -> [512,512]
    k = x_k @ WK
    v = x_v @ WV
    y = softmax(q @ k, axis=-1) @ v     -> [512,512]

Strategy (v2): data-parallel over the leading N axis (64 rows/core x 8
cores).  The host pre-transposes each core's x slabs to d-major fp16
([256, 32768]), so the rank-1 projections run entirely on the tensor
engine: each [128 d, 128 rows] chunk is loaded as the stationary lhsT
and multiplied by the W-half [128, 1] moving operand, producing one
fp32 PSUM column per chunk (~30 ns each measured).  DVE/GpSimd do no
bulk work; per-core HBM traffic halves vs fp32 (48 MB -> ~140 us DMA
floor, the roofline).  k/v projections are re-tiled on-chip ([i, m]
rows), AllGathered in fp16, and the tiny attention tail runs fp16 on
the PE with fp32 PSUM accumulation.
"""

import numpy as np

import concourse.bass as bass
import concourse.mybir as mybir
import concourse.tile as tile
from concourse import bacc
from concourse.bass_utils import run_bass_kernel_spmd
from concourse.masks import make_identity

N = 512          # attention size (rows/cols)
D = 256          # projection dim
CORES = 8
NL = N // CORES  # 64 leading rows per core
R = NL * N       # 32768 projection rows per tensor per core
RNG = 4096       # rows per DMA tile ([128, RNG] fp16 = 1 MB)
NRG = R // RNG   # 8 ranges per tensor
CPT = RNG // 128  # 32 chunks of 128 rows per tile

F32 = mybir.dt.float32
F16 = mybir.dt.float16

_CACHE = {}


def _build():
    if "nc" in _CACHE:
        return _CACHE["nc"]

    nc = bacc.Bacc(
        "TRN2", target_bir_lowering=False, debug=False, num_devices=CORES
    )

    xkt = nc.dram_tensor("xkt", [D, R], F16, kind="ExternalInput")
    xvt = nc.dram_tensor("xvt", [D, R], F16, kind="ExternalInput")
    xqt = nc.dram_tensor("xqt", [D, R], F16, kind="ExternalInput")
    wall = nc.dram_tensor("wall", [128, 6], F16, kind="ExternalInput")
    yout = nc.dram_tensor("yout", [NL, N], F32, kind="ExternalOutput")

    with tile.TileContext(nc) as tc:
        with (
            tc.tile_pool(name="consts", bufs=1) as consts,
            tc.tile_pool(name="xs", bufs=6) as xs_pool,
            tc.tile_pool(name="psum", bufs=1, space="PSUM") as psum_pool,
            tc.tile_pool(name="dram", bufs=1, space="DRAM") as dram_pool,
        ):
            w_t = consts.tile([128, 6], F16)
            nc.gpsimd.dma_start(w_t[:], wall[:])
            ident = consts.tile([128, 128], F32)
            make_identity(nc, ident[:])

            # fp32 psum accumulators, [b%128, (b//128)*64 + i] layout:
            # ps[p, bb*64 + a] = proj value of slab row a*512 + bb*128 + p
            ps = {
                t: psum_pool.tile([128, 4 * NL], F32, tag=f"ps{t}", name=f"ps{t}")
                for t in ("k", "v", "q")
            }

            rings = [nc.sync, nc.scalar, nc.gpsimd]

            def project(x_dram, widx, dest):
                for rg in range(NRG):
                    tiles = []
                    for h in (0, 1):
                        xt = xs_pool.tile([128, RNG], F16, tag="xt", name="xt")
                        # rotate dma rings to hide per-DMA latency gaps
                        ring = rings[(rg * 2 + h) % 3]
                        ring.dma_start(
                            xt[:],
                            x_dram[h * 128 : (h + 1) * 128,
                                   rg * RNG : (rg + 1) * RNG],
                        )
                        tiles.append(xt)
                    for j in range(CPT):
                        # slab rows rg*RNG + j*128 ... +128:
                        # a = rg*(RNG//512) + j//4, b-block bb = j%4
                        # ->  psum column bb*64 + a
                        col = (j % 4) * NL + rg * (RNG // N) + j // 4
                        for h in (0, 1):
                            nc.tensor.matmul(
                                dest[:, col : col + 1],
                                lhsT=tiles[h][:, j * 128 : (j + 1) * 128],
                                rhs=w_t[:, 2 * widx + h : 2 * widx + h + 1],
                                start=(h == 0),
                                stop=(h == 1),
                            )

            # re-tile a projection psum [128, 256] into [a, b] rows (fp16)
            def pack_rows(src_ps, dst, dst_off):
                sbt = consts.tile([128, 4 * NL], F32, name=f"sbt{dst_off}")
                nc.scalar.activation(
                    sbt[:], src_ps[:], mybir.ActivationFunctionType.Copy
                )
                for bb in range(4):
                    pt = psum_pool.tile([NL, 128], F32, tag="tp", bufs=1, name="pt")
                    nc.tensor.transpose(
                        pt[:], sbt[:, bb * NL : (bb + 1) * NL], ident[:]
                    )
                    nc.vector.tensor_copy(
                        out=dst[:, dst_off + bb * 128 : dst_off + (bb + 1) * 128],
                        in_=pt[:],
                    )

            # ---- k then v, each gathered right away so the collective
            # overlaps the remaining x streams ----
            kv_loc = consts.tile([NL, 2 * N], F16)
            cc_in_k = dram_pool.tile([NL, N], F16)
            cc_in_v = dram_pool.tile([NL, N], F16)
            cc_out_k = dram_pool.tile([N, N], F16, addr_space="Shared")
            cc_out_v = dram_pool.tile([N, N], F16, addr_space="Shared")

            project(xkt, 1, ps["k"])
            pack_rows(ps["k"], kv_loc, 0)
            nc.gpsimd.dma_start(cc_in_k[:], kv_loc[:, 0:N])
            nc.gpsimd.collective_compute(
                "AllGather",
                mybir.AluOpType.bypass,
                replica_groups=[list(range(CORES))],
                ins=[cc_in_k[:].opt()],
                outs=[cc_out_k[:].opt()],
            )

            project(xvt, 2, ps["v"])
            pack_rows(ps["v"], kv_loc, N)
            nc.gpsimd.dma_start(cc_in_v[:], kv_loc[:, N : 2 * N])
            nc.gpsimd.collective_compute(
                "AllGather",
                mybir.AluOpType.bypass,
                replica_groups=[list(range(CORES))],
                ins=[cc_in_v[:].opt()],
                outs=[cc_out_v[:].opt()],
            )

            # ---- q projection (overlaps with the AllGather) ----
            project(xqt, 0, ps["q"])
            # q stays in [m%128, (m//128)*64 + i] layout: lhsT blocks for the
            # qk matmul are direct [64, 64] slices of it
            q_sbT = consts.tile([128, 4 * NL], F16)
            nc.scalar.activation(
                q_sbT[:], ps["q"][:], mybir.ActivationFunctionType.Copy
            )

            # gathered k/v rows, two ranks per [128, N] tile:
            # k_sb[b][64*(r%2) + m_local, j] = k row of rank r = 2b + (r%2)
            k_sb = [consts.tile([128, N], F16, name=f"ksb{b}") for b in range(4)]
            v_sb = [consts.tile([128, N], F16, name=f"vsb{b}") for b in range(4)]
            for b in range(4):
                nc.gpsimd.dma_start(
                    k_sb[b][:], cc_out_k[b * 128 : (b + 1) * 128, :]
                )
                nc.gpsimd.dma_start(
                    v_sb[b][:], cc_out_v[b * 128 : (b + 1) * 128, :]
                )

            # ---- attention tail ----
            # q_sbT[:, b*64:(b+1)*64] is q[i, m] transposed for m-block b
            # (128 m rows = gathered ranks 2b, 2b+1) -> 4 full-K matmuls
            py = psum_pool.tile([NL, N], F32, tag="mm", name="py")
            for b in range(4):
                nc.tensor.matmul(
                    py[:], lhsT=q_sbT[:, b * NL : (b + 1) * NL], rhs=k_sb[b][:],
                    start=(b == 0), stop=(b == 3),
                )

            neg_mx = consts.tile([NL, 1], F32)
            nc.vector.tensor_reduce(
                out=neg_mx[:], in_=py[:], axis=mybir.AxisListType.X,
                op=mybir.AluOpType.max, negate=True,
            )
            s_sb = consts.tile([NL, N], F32)
            sumexp = consts.tile([NL, 1], F32)
            nc.scalar.activation(
                s_sb[:], py[:], mybir.ActivationFunctionType.Exp,
                bias=neg_mx[:], scale=1.0, accum_out=sumexp[:],
            )
            rsum = consts.tile([NL, 1], F32)
            nc.vector.reciprocal(rsum[:], sumexp[:])

            # st2[b]: transposed softmax blocks for ranks 2b, 2b+1 stacked
            st2 = [consts.tile([128, NL], F16, name=f"st{b}") for b in range(4)]
            for b in range(4):
                for half in (0, 1):
                    r = 2 * b + half
                    pt2 = psum_pool.tile([NL, NL], F32, tag="tp2", bufs=1, name="pt2")
                    nc.tensor.transpose(
                        pt2[:],
                        s_sb[:, r * NL : (r + 1) * NL],
                        ident[:NL, :NL],
                    )
                    nc.vector.tensor_copy(
                        out=st2[b][NL * half : NL * half + NL, :], in_=pt2[:]
                    )

            po = psum_pool.tile([NL, N], F32, tag="mm2", name="po")
            for b in range(4):
                nc.tensor.matmul(
                    po[:], lhsT=st2[b][:], rhs=v_sb[b][:],
                    start=(b == 0), stop=(b == 3),
                )

            out_sb = consts.tile([NL, N], F32)
            nc.vector.tensor_scalar_mul(out_sb[:], po[:], rsum[:])
            nc.sync.dma_start(yout[:], out_sb[:])

    nc.compile()
    _CACHE["nc"] = nc
    return nc


def _make_in_maps(inputs):
    x_q = np.asarray(inputs["x_q"], dtype=np.float32)
    x_k = np.asarray(inputs["x_k"], dtype=np.float32)
    x_v = np.asarray(inputs["x_v"], dtype=np.float32)
    w_all = np.stack(
        [
            np.asarray(inputs["WQ"], dtype=np.float32),
            np.asarray(inputs["WK"], dtype=np.float32),
            np.asarray(inputs["WV"], dtype=np.float32),
        ],
        axis=1,
    ).reshape(2, 128, 3).transpose(1, 2, 0).reshape(128, 6)  # [p, 2*tensor+half]
    w_all = np.ascontiguousarray(w_all).astype(np.float16)
    in_maps = []
    for r in range(CORES):
        sl = slice(r * NL, (r + 1) * NL)
        in_maps.append(
            {
                "xqt": x_q[sl].reshape(R, D).T.astype(np.float16),
                "xkt": x_k[sl].reshape(R, D).T.astype(np.float16),
                "xvt": x_v[sl].reshape(R, D).T.astype(np.float16),
                "wall": w_all,
            }
        )
    return in_maps


def _run(inputs, trace=False):
    nc = _build()
    res = run_bass_kernel_spmd(
        nc, _make_in_maps(inputs), core_ids=list(range(CORES)), trace=trace
    )
    out = np.concatenate(
        [res.results[r]["yout"] for r in range(CORES)], axis=0
    ).astype(np.float32)
    return out, res


def kernel(**inputs):
    out, _ = _run(inputs)
    return out


# revision 23
# speedup vs baseline: 1.1843x; 1.1011x over previous
"""Trainium2 Bass kernel for rank-1-projection attention.

Computation (reference, fp32):
    q = x_q @ WQ            [512,512,256]@[256] -> [512,512]
    k = x_k @ WK
    v = x_v @ WV
    y = softmax(q @ k, axis=-1) @ v     -> [512,512]

Strategy (v2): data-parallel over the leading N axis (64 rows/core x 8
cores).  The host pre-transposes each core's x slabs to d-major fp16
([256, 32768]), so the rank-1 projections run entirely on the tensor
engine: each [128 d, 128 rows] chunk is loaded as the stationary lhsT
and multiplied by the W-half [128, 1] moving operand, producing one
fp32 PSUM column per chunk (~30 ns each measured).  DVE/GpSimd do no
bulk work; per-core HBM traffic halves vs fp32 (48 MB -> ~140 us DMA
floor, the roofline).  k/v projections are re-tiled on-chip ([i, m]
rows), AllGathered in fp16, and the tiny attention tail runs fp16 on
the PE with fp32 PSUM accumulation.
"""

import numpy as np

import concourse.bass as bass
import concourse.mybir as mybir
import concourse.tile as tile
from concourse import bacc
from concourse.bass_utils import run_bass_kernel_spmd
from concourse.masks import make_identity

N = 512          # attention size (rows/cols)
D = 256          # projection dim
CORES = 8
NL = N // CORES  # 64 leading rows per core
R = NL * N       # 32768 projection rows per tensor per core
RNG = 4096       # rows per DMA tile ([128, RNG] fp16 = 1 MB)
NRG = R // RNG   # 8 ranges per tensor
CPT = RNG // 128  # 32 chunks of 128 rows per tile

F32 = mybir.dt.float32
F16 = mybir.dt.float16

_CACHE = {}


def _build():
    if "nc" in _CACHE:
        return _CACHE["nc"]

    nc = bacc.Bacc(
        "TRN2", target_bir_lowering=False, debug=False, num_devices=CORES
    )

    xkt = nc.dram_tensor("xkt", [D, R], F16, kind="ExternalInput")
    xvt = nc.dram_tensor("xvt", [D, R], F16, kind="ExternalInput")
    xqt = nc.dram_tensor("xqt", [D, R], F16, kind="ExternalInput")
    wall = nc.dram_tensor("wall", [128, 6], F16, kind="ExternalInput")
    yout = nc.dram_tensor("yout", [NL, N], F32, kind="ExternalOutput")

    with tile.TileContext(nc) as tc:
        with (
            tc.tile_pool(name="consts", bufs=1) as consts,
            tc.tile_pool(name="xs", bufs=6) as xs_pool,
            tc.tile_pool(name="psum", bufs=1, space="PSUM") as psum_pool,
            tc.tile_pool(name="dram", bufs=1, space="DRAM") as dram_pool,
        ):
            w_t = consts.tile([128, 6], F16)
            nc.gpsimd.dma_start(w_t[:], wall[:])
            ident = consts.tile([128, 128], F32)
            make_identity(nc, ident[:])

            # fp32 psum accumulators, [b%128, (b//128)*64 + i] layout:
            # ps[p, bb*64 + a] = proj value of slab row a*512 + bb*128 + p
            ps = {
                t: psum_pool.tile([128, 4 * NL], F32, tag=f"ps{t}", name=f"ps{t}")
                for t in ("k", "v", "q")
            }

            rings = [nc.sync, nc.scalar, nc.gpsimd]

            def project(x_dram, widx, dest):
                for rg in range(NRG):
                    tiles = []
                    for h in (0, 1):
                        xt = xs_pool.tile([128, RNG], F16, tag="xt", name="xt")
                        # alternate hwdge rings to hide per-DMA latency gaps
                        ring = rings[h]
                        ring.dma_start(
                            xt[:],
                            x_dram[h * 128 : (h + 1) * 128,
                                   rg * RNG : (rg + 1) * RNG],
                        )
                        tiles.append(xt)
                    for j in range(CPT):
                        # slab rows rg*RNG + j*128 ... +128:
                        # a = rg*(RNG//512) + j//4, b-block bb = j%4
                        # ->  psum column bb*64 + a
                        col = (j % 4) * NL + rg * (RNG // N) + j // 4
                        for h in (0, 1):
                            nc.tensor.matmul(
                                dest[:, col : col + 1],
                                lhsT=tiles[h][:, j * 128 : (j + 1) * 128],
                                rhs=w_t[:, 2 * widx + h : 2 * widx + h + 1],
                                start=(h == 0),
                                stop=(h == 1),
                            )

            # re-tile a projection psum [128, 256] into [a, b] rows (fp16)
            def pack_rows(src_ps, dst, dst_off):
                sbt = consts.tile([128, 4 * NL], F32, name=f"sbt{dst_off}")
                nc.scalar.activation(
                    sbt[:], src_ps[:], mybir.ActivationFunctionType.Copy
                )
                for bb in range(4):
                    pt = psum_pool.tile([NL, 128], F32, tag="tp", bufs=1, name="pt")
                    nc.tensor.transpose(
                        pt[:], sbt[:, bb * NL : (bb + 1) * NL], ident[:]
                    )
                    nc.vector.tensor_copy(
                        out=dst[:, dst_off + bb * 128 : dst_off + (bb + 1) * 128],
                        in_=pt[:],
                    )

            # ---- k then v, each gathered right away so the collective
            # overlaps the remaining x streams ----
            kv_loc = consts.tile([NL, 2 * N], F16)
            cc_in_k = dram_pool.tile([NL, N], F16)
            cc_in_v = dram_pool.tile([NL, N], F16)
            cc_out_k = dram_pool.tile([N, N], F16, addr_space="Shared")
            cc_out_v = dram_pool.tile([N, N], F16, addr_space="Shared")

            project(xkt, 1, ps["k"])
            pack_rows(ps["k"], kv_loc, 0)
            nc.gpsimd.dma_start(cc_in_k[:], kv_loc[:, 0:N])
            nc.gpsimd.collective_compute(
                "AllGather",
                mybir.AluOpType.bypass,
                replica_groups=[list(range(CORES))],
                ins=[cc_in_k[:].opt()],
                outs=[cc_out_k[:].opt()],
            )

            project(xvt, 2, ps["v"])
            pack_rows(ps["v"], kv_loc, N)
            nc.gpsimd.dma_start(cc_in_v[:], kv_loc[:, N : 2 * N])
            nc.gpsimd.collective_compute(
                "AllGather",
                mybir.AluOpType.bypass,
                replica_groups=[list(range(CORES))],
                ins=[cc_in_v[:].opt()],
                outs=[cc_out_v[:].opt()],
            )

            # ---- q projection (overlaps with the AllGather) ----
            project(xqt, 0, ps["q"])
            # q stays in [m%128, (m//128)*64 + i] layout: lhsT blocks for the
            # qk matmul are direct [64, 64] slices of it
            q_sbT = consts.tile([128, 4 * NL], F16)
            nc.scalar.activation(
                q_sbT[:], ps["q"][:], mybir.ActivationFunctionType.Copy
            )

            # gathered k/v rows, two ranks per [128, N] tile:
            # k_sb[b][64*(r%2) + m_local, j] = k row of rank r = 2b + (r%2)
            k_sb = [consts.tile([128, N], F16, name=f"ksb{b}") for b in range(4)]
            v_sb = [consts.tile([128, N], F16, name=f"vsb{b}") for b in range(4)]
            for b in range(4):
                nc.gpsimd.dma_start(
                    k_sb[b][:], cc_out_k[b * 128 : (b + 1) * 128, :]
                )
                nc.gpsimd.dma_start(
                    v_sb[b][:], cc_out_v[b * 128 : (b + 1) * 128, :]
                )

            # ---- attention tail ----
            # q_sbT[:, b*64:(b+1)*64] is q[i, m] transposed for m-block b
            # (128 m rows = gathered ranks 2b, 2b+1) -> 4 full-K matmuls
            py = psum_pool.tile([NL, N], F32, tag="mm", name="py")
            for b in range(4):
                nc.tensor.matmul(
                    py[:], lhsT=q_sbT[:, b * NL : (b + 1) * NL], rhs=k_sb[b][:],
                    start=(b == 0), stop=(b == 3),
                )

            neg_mx = consts.tile([NL, 1], F32)
            nc.vector.tensor_reduce(
                out=neg_mx[:], in_=py[:], axis=mybir.AxisListType.X,
                op=mybir.AluOpType.max, negate=True,
            )
            s_sb = consts.tile([NL, N], F32)
            sumexp = consts.tile([NL, 1], F32)
            nc.scalar.activation(
                s_sb[:], py[:], mybir.ActivationFunctionType.Exp,
                bias=neg_mx[:], scale=1.0, accum_out=sumexp[:],
            )
            rsum = consts.tile([NL, 1], F32)
            nc.vector.reciprocal(rsum[:], sumexp[:])

            # st2[b] = s[:, b*128:(b+1)*128].T  (one [64,128] transpose each)
            st2 = [consts.tile([128, NL], F16, name=f"st{b}") for b in range(4)]
            for b in range(4):
                pt2 = psum_pool.tile([128, NL], F32, tag="tp2", bufs=2, name="pt2")
                nc.tensor.transpose(
                    pt2[:], s_sb[:, b * 128 : (b + 1) * 128], ident[:NL, :NL]
                )
                nc.vector.tensor_copy(out=st2[b][:], in_=pt2[:])

            po = psum_pool.tile([NL, N], F32, tag="mm2", name="po")
            for b in range(4):
                nc.tensor.matmul(
                    po[:], lhsT=st2[b][:], rhs=v_sb[b][:],
                    start=(b == 0), stop=(b == 3),
                )

            out_sb = consts.tile([NL, N], F32)
            nc.vector.tensor_scalar_mul(out_sb[:], po[:], rsum[:])
            nc.sync.dma_start(yout[:], out_sb[:])

    nc.compile()
    _CACHE["nc"] = nc
    return nc


def _make_in_maps(inputs):
    x_q = np.asarray(inputs["x_q"], dtype=np.float32)
    x_k = np.asarray(inputs["x_k"], dtype=np.float32)
    x_v = np.asarray(inputs["x_v"], dtype=np.float32)
    w_all = np.stack(
        [
            np.asarray(inputs["WQ"], dtype=np.float32),
            np.asarray(inputs["WK"], dtype=np.float32),
            np.asarray(inputs["WV"], dtype=np.float32),
        ],
        axis=1,
    ).reshape(2, 128, 3).transpose(1, 2, 0).reshape(128, 6)  # [p, 2*tensor+half]
    w_all = np.ascontiguousarray(w_all).astype(np.float16)
    in_maps = []
    for r in range(CORES):
        sl = slice(r * NL, (r + 1) * NL)
        in_maps.append(
            {
                "xqt": x_q[sl].reshape(R, D).T.astype(np.float16),
                "xkt": x_k[sl].reshape(R, D).T.astype(np.float16),
                "xvt": x_v[sl].reshape(R, D).T.astype(np.float16),
                "wall": w_all,
            }
        )
    return in_maps


def _run(inputs, trace=False):
    nc = _build()
    res = run_bass_kernel_spmd(
        nc, _make_in_maps(inputs), core_ids=list(range(CORES)), trace=trace
    )
    out = np.concatenate(
        [res.results[r]["yout"] for r in range(CORES)], axis=0
    ).astype(np.float32)
    return out, res


def kernel(**inputs):
    out, _ = _run(inputs)
    return out
